# revision 45
# baseline (speedup 1.0000x reference)
"""Trainium2 Bass kernel: GQA attention block (q/k/v proj + RoPE + causal
attention + out proj), sharded over 8 NeuronCores as DP2 (batch) x TP4 (heads).

Per core (batch b = core//4, shard tp = core%4): 2048 tokens, 8 q heads,
2 kv heads. Device computes a row-parallel partial of the output projection;
the host sums the 4 TP partials per batch.

Layout choices (all chosen so attention needs no on-chip transposes of Q/K/P):
  - X^T [hidden, tok] streamed from host (host transposes; free).
  - Q^T [d, tok] / K^T [d, tok] produced directly by the projection matmuls
    (weights as lhsT, X^T as rhs), RoPE applied in this layout, then written
    into zero-padded per-kv copies (rows 64:128 = 0) so the scores matmuls
    contract K=128 (sustained 64-row PE mode trips the HAM clock gate).
  - scores_T [t, s] = (K^T tile).T @ Q^T  -> softmax denominator comes from an
    appended ones-column on V (fused into the AV matmul); no max-subtraction
    (scores are O(5), exp is safe in fp32).
  - AV produces out^T [d, s] == A^T, which is exactly the lhsT the row-parallel
    o_proj needs.
  - Attention for head-pair j is emitted right after Q-projection group j, so
    the scalar-engine exp stream (the attention-phase pacer) starts during the
    projection phase instead of after it.
"""

import numpy as np
import ml_dtypes
from contextlib import ExitStack

import concourse.bass as bass
import concourse.tile as tile
from concourse import bacc, mybir
from concourse.masks import make_identity
import concourse.bass_utils as bass_utils

P = 128
S = 2048          # tokens per core (one batch element)
HID = 2048
D = 64            # head dim
NQ = 512          # 8 local q heads * 64
NKV = 128         # 2 local kv heads * 64
SW = 1024         # attention s-swath width
N_CORES = 8
BF16 = mybir.dt.bfloat16
F32 = mybir.dt.float32
AFT = mybir.ActivationFunctionType
BF = ml_dtypes.bfloat16


def _segs(sl, hi):
    """Split [sl, hi) at 512 boundaries (one PSUM bank per matmul)."""
    bounds = sorted({sl, hi} | {b for b in range(512, hi, 512) if b > sl})
    return list(zip(bounds[:-1], bounds[1:]))


def build():
    nc = bacc.Bacc("TRN2", target_bir_lowering=False, debug=False, num_devices=N_CORES)
    xt_d = nc.declare_dram_parameter("xt", [16, P, S], BF16, isOutput=False)
    wq_d = nc.declare_dram_parameter("wq", [4, P, 16 * P], BF16, isOutput=False)
    wk_d = nc.declare_dram_parameter("wk", [16, P, NKV], BF16, isOutput=False)
    wv_d = nc.declare_dram_parameter("wv", [16, P, NKV], BF16, isOutput=False)
    wo_d = nc.declare_dram_parameter("wo", [4, P, S], BF16, isOutput=False)
    cos_d = nc.declare_dram_parameter("cos2", [P, S], BF16, isOutput=False)
    sin_d = nc.declare_dram_parameter("sin2", [P, S], BF16, isOutput=False)
    mask_d = nc.declare_dram_parameter("mask", [P, P], BF16, isOutput=False)
    out_d = nc.declare_dram_parameter("out", [S, HID], F32, isOutput=True)

    with tile.TileContext(nc) as tc, ExitStack() as ctx:
        const = ctx.enter_context(tc.tile_pool(name="const", bufs=1))
        big = ctx.enter_context(tc.tile_pool(name="big", bufs=1))

        # persistent across phases
        qtz_sb = [big.tile([P, 4, S], BF16, tag=f"qtz{kv}", name=f"qtz{kv}")
                  for kv in range(2)]
        ktz_sb = [big.tile([P, S], BF16, tag=f"ktz{kv}", name=f"ktz{kv}")
                  for kv in range(2)]
        vag_sb = big.tile([P, 16, 130], BF16)    # V tiles + ones col per kv head
        at_sb = big.tile([P, 4, S], BF16)        # A^T (normalized attention out)

        cos_sb = const.tile([P, S], BF16)
        sin_sb = const.tile([P, S], BF16)
        mask_sb = const.tile([P, P], BF16)
        ident = const.tile([P, P], BF16)

        nc.sync.dma_start(cos_sb[:], cos_d[:])
        nc.sync.dma_start(sin_sb[:], sin_d[:])
        nc.sync.dma_start(mask_sb[:], mask_d[:])
        make_identity(nc, ident)
        nc.vector.memset(vag_sb[:, :, 64:65], 1.0)
        nc.vector.memset(vag_sb[:, :, 129:130], 1.0)
        for kv in range(2):
            nc.vector.memset(qtz_sb[kv][64:128, :, :], 0.0)
            nc.vector.memset(ktz_sb[kv][64:128, :], 0.0)

        # ------------- Phases 1+2 interleaved: QKV^T + RoPE + attention -------------
        with tc.tile_pool(name="ph1", bufs=1) as ph1, \
             tc.tile_pool(name="wqp", bufs=2) as wqp, \
             tc.tile_pool(name="rtmp", bufs=3) as rtmp, \
             tc.tile_pool(name="exp", bufs=12) as p_ex, \
             tc.tile_pool(name="nrm", bufs=1) as p_nrm, \
             tc.tile_pool(name="p_mm1", bufs=2, space="PSUM") as p_mm1, \
             tc.tile_pool(name="p_sc", bufs=2, space="PSUM") as p_sc, \
             tc.tile_pool(name="p_out", bufs=1, space="PSUM") as p_out:
            xt_sb = ph1.tile([P, 16, S], BF16)
            wk_sb = ph1.tile([P, 16, NKV], BF16)
            wv_sb = ph1.tile([P, 16, NKV], BF16)
            vt_sb = ph1.tile([P, S], BF16)       # V^T staging
            for i in range(16):
                nc.sync.dma_start(xt_sb[:, i, :], xt_d[i])
                nc.sync.dma_start(wk_sb[:, i, :], wk_d[i])
                nc.sync.dma_start(wv_sb[:, i, :], wv_d[i])

            def rope(ps, sw):
                # returns rope(ps) as a transient bf16 tile; ps [P,512] f32 PSUM
                cs = cos_sb[:, sw * 512:(sw + 1) * 512]
                sn = sin_sb[:, sw * 512:(sw + 1) * 512]
                q = rtmp.tile([P, 512], BF16, tag="ropeq")
                nc.scalar.copy(q[:], ps[:])
                tmp = rtmp.tile([P, 512], BF16, tag="ropet")
                for b0 in (0, 64):
                    nc.vector.tensor_copy(tmp[b0:b0 + 32, :], q[b0 + 32:b0 + 64, :])
                    nc.vector.tensor_copy(tmp[b0 + 32:b0 + 64, :], q[b0:b0 + 32, :])
                ro = rtmp.tile([P, 512], BF16, tag="ropeo")
                nc.vector.tensor_mul(ro[:], q[:], cs)
                nc.vector.tensor_mul(tmp[:], tmp[:], sn)
                nc.vector.tensor_add(ro[:], ro[:], tmp[:])
                return ro

            for sw in range(4):         # K^T
                ps = p_mm1.tile([P, 512], F32, tag="mm")
                for i in range(16):
                    nc.tensor.matmul(ps[:], wk_sb[:, i, :],
                                     xt_sb[:, i, sw * 512:(sw + 1) * 512],
                                     start=(i == 0), stop=(i == 15))
                ro = rope(ps, sw)
                for kv in range(2):
                    nc.vector.tensor_copy(ktz_sb[kv][0:64, sw * 512:(sw + 1) * 512],
                                          ro[64 * kv:64 * kv + 64, :])
            for sw in range(4):         # V^T
                ps = p_mm1.tile([P, 512], F32, tag="mm")
                for i in range(16):
                    nc.tensor.matmul(ps[:], wv_sb[:, i, :],
                                     xt_sb[:, i, sw * 512:(sw + 1) * 512],
                                     start=(i == 0), stop=(i == 15))
                nc.vector.tensor_copy(vt_sb[:, sw * 512:(sw + 1) * 512], ps[:])
            for tt in range(16):        # V^T -> V tiles (PE transpose)
                pt = p_mm1.tile([P, P], BF16, tag="mm")
                nc.tensor.transpose(pt[:], vt_sb[:, tt * 128:(tt + 1) * 128], ident[:])
                nc.vector.tensor_copy(vag_sb[:, tt, 0:64], pt[:, 0:64])
                nc.vector.tensor_copy(vag_sb[:, tt, 65:129], pt[:, 64:128])

            for j in range(4):          # Q^T block j, then attention for head-pair j
                wq_j = wqp.tile([P, 16, P], BF16, tag="wqj")
                nc.sync.dma_start(wq_j[:], wq_d[j])
                for sw in range(4):
                    ps = p_mm1.tile([P, 512], F32, tag="mm")
                    for i in range(16):
                        nc.tensor.matmul(ps[:], wq_j[:, i, :],
                                         xt_sb[:, i, sw * 512:(sw + 1) * 512],
                                         start=(i == 0), stop=(i == 15))
                    ro = rope(ps, sw)
                    for kv in range(2):
                        nc.vector.tensor_copy(
                            qtz_sb[kv][0:64, j, sw * 512:(sw + 1) * 512],
                            ro[64 * kv:64 * kv + 64, :])

                for kv in range(2):     # attention for (kv, head j)
                    pb = 64 * kv
                    vo = 65 * kv
                    for swh in range(2):
                        s0 = SW * swh
                        n_t = s0 // P + 8
                        po = p_out.tile([65, SW], F32, tag="po")
                        for tt in range(n_t):
                            t0 = P * tt
                            sl = max(0, t0 - s0)
                            sc = p_sc.tile([P, SW], F32, tag="sc")
                            for a, b in _segs(sl, SW):
                                nc.tensor.matmul(sc[:, a:b], ktz_sb[kv][:, t0:t0 + P],
                                                 qtz_sb[kv][:, j, s0 + a:s0 + b],
                                                 start=True, stop=True)
                            ex = p_ex.tile([P, SW], BF16, tag="ex")
                            nc.scalar.activation(ex[:, sl:SW], sc[:, sl:SW], AFT.Exp)
                            if t0 >= s0:  # diagonal block: zero the t>s triangle
                                nc.gpsimd.tensor_mul(ex[:, sl:sl + P],
                                                     ex[:, sl:sl + P], mask_sb[:])
                            for a, b in _segs(sl, SW):
                                nc.tensor.matmul(po[:, a:b], vag_sb[:, tt, vo:vo + 65],
                                                 ex[:, a:b], start=(tt == 0),
                                                 stop=(tt == n_t - 1),
                                                 skip_group_check=True)
                        # one ACT copy evicts po -> SBUF so the PSUM bank frees
                        # immediately; the normalize chain then runs SBUF-side
                        pos = p_nrm.tile([65, SW], F32, tag="pos")
                        nc.vector.tensor_copy(pos[:], po[:])
                        den = p_nrm.tile([1, SW], F32, tag="den")
                        nc.vector.tensor_copy(den[:], pos[64:65, :])
                        rec = p_nrm.tile([1, SW], F32, tag="rec")
                        nc.vector.reciprocal_approx_fast(rec[:], den[:])
                        bc = p_nrm.tile([64, SW], F32, tag="bc")
                        nc.gpsimd.partition_broadcast(bc[:], rec[:], channels=64)
                        nc.vector.tensor_mul(at_sb[pb:pb + 64, j, s0:s0 + SW],
                                             pos[0:64, :], bc[:])

        # ---------------- Phase 3: o_proj (row-parallel partial) ----------------
        with tc.tile_pool(name="wop", bufs=3) as wop, \
             tc.tile_pool(name="oev", bufs=6) as p_oev, \
             tc.tile_pool(name="p_mm3", bufs=2, space="PSUM") as p_mm3:
            for nsw in range(4):
                wo_c = wop.tile([P, 4, 512], BF16, tag="woc")
                for j in range(4):
                    nc.sync.dma_start(wo_c[:, j, :], wo_d[j, :, nsw * 512:(nsw + 1) * 512])
                for tt in range(16):
                    ps = p_mm3.tile([P, 512], F32, tag="mm")
                    for j in range(4):
                        nc.tensor.matmul(ps[:], at_sb[:, j, tt * 128:(tt + 1) * 128],
                                         wo_c[:, j, :], start=(j == 0), stop=(j == 3))
                    ov = p_oev.tile([P, 512], F32, tag="ov")
                    if (tt * 4 + nsw) % 2 == 0:
                        nc.scalar.copy(ov[:], ps[:])
                    else:
                        nc.vector.tensor_copy(ov[:], ps[:])
                    nc.sync.dma_start(out_d[tt * 128:(tt + 1) * 128,
                                            nsw * 512:(nsw + 1) * 512], ov[:])

    nc.compile()
    return nc


_CACHE = {}


def _get_nc():
    if "nc" not in _CACHE:
        _CACHE["nc"] = build()
    return _CACHE["nc"]


def make_in_maps(hidden_states, Wq, Wk, Wv, Wo):
    """Host-side shard + layout prep. Returns one input map per core."""
    hs = np.asarray(hidden_states, dtype=np.float32)
    Wq = np.asarray(Wq, dtype=np.float32)
    Wk = np.asarray(Wk, dtype=np.float32)
    Wv = np.asarray(Wv, dtype=np.float32)
    Wo = np.asarray(Wo, dtype=np.float32)
    scale = D ** -0.5

    # RoPE tables, transposed ([d, t]); cos duplicated, sin signed (-,+)
    inv = 1.0 / (10000.0 ** (np.arange(0, D, 2, dtype=np.float32) / D))  # [32]
    fr = np.outer(np.arange(S, dtype=np.float32), inv)                   # [S, 32]
    cosh, sinh = np.cos(fr).T, np.sin(fr).T                              # [32, S]
    cos64 = np.concatenate([cosh, cosh], 0)                              # [64, S]
    sin64 = np.concatenate([-sinh, sinh], 0)
    cos2 = np.concatenate([cos64, cos64], 0).astype(BF)                  # [128, S]
    sin2 = np.concatenate([sin64, sin64], 0).astype(BF)
    mask = np.triu(np.ones((P, P), dtype=np.float32)).astype(BF)         # keep t <= s

    perm = [0, 4, 1, 5, 2, 6, 3, 7]  # interleave kv0/kv1 q heads per 128-block
    in_maps = []
    for c in range(N_CORES):
        b, tp = divmod(c, 4)
        xt = np.ascontiguousarray(hs[b].T).astype(BF).reshape(16, P, S)
        wq = (Wq[:, tp * NQ:(tp + 1) * NQ] * scale).reshape(HID, 8, D)
        wq = np.ascontiguousarray(wq[:, perm, :]).reshape(16, P, 4, P)
        # [4, 128, 16*128]: per head-pair j, lhsT tiles for all 16 h-blocks
        wq = np.ascontiguousarray(wq.transpose(2, 1, 0, 3)).reshape(4, P, 16 * P).astype(BF)
        wk = np.ascontiguousarray(Wk[:, tp * NKV:(tp + 1) * NKV]).astype(BF).reshape(16, P, NKV)
        wv = np.ascontiguousarray(Wv[:, tp * NKV:(tp + 1) * NKV]).astype(BF).reshape(16, P, NKV)
        wo = Wo[tp * NQ:(tp + 1) * NQ, :].reshape(8, D, HID)
        wo = np.ascontiguousarray(wo[perm]).reshape(4, P, HID).astype(BF)
        in_maps.append({"xt": xt, "wq": wq, "wk": wk, "wv": wv, "wo": wo,
                        "cos2": cos2, "sin2": sin2, "mask": mask})
    return in_maps


def kernel(hidden_states, Wq, Wk, Wv, Wo):
    nc = _get_nc()
    in_maps = make_in_maps(hidden_states, Wq, Wk, Wv, Wo)
    res = bass_utils.run_bass_kernel_spmd(nc, in_maps, list(range(N_CORES)))
    _CACHE["last_results"] = res
    parts = [res.results[c]["out"] for c in range(N_CORES)]
    out = np.stack([parts[0] + parts[1] + parts[2] + parts[3],
                    parts[4] + parts[5] + parts[6] + parts[7]], axis=0)
    return out.astype(np.float32)


# revision 46
# speedup vs baseline: 1.5343x; 1.5343x over previous
"""Trainium2 Bass kernel: GQA attention block (q/k/v proj + RoPE + causal
attention + out proj), sharded over 8 NeuronCores as DP2 (batch) x TP4 (heads).

Per core (batch b = core//4, shard tp = core%4): 2048 tokens, 8 q heads,
2 kv heads. Device computes a row-parallel partial of the output projection;
the host sums the 4 TP partials per batch.

Layout choices (all chosen so attention needs no on-chip transposes of Q/K/P):
  - X^T [hidden, tok] streamed from host (host transposes; free).
  - Q^T [d, tok] / K^T [d, tok] produced directly by the projection matmuls
    (weights as lhsT, X^T as rhs), RoPE applied in this layout, then written
    into zero-padded per-kv copies (rows 64:128 = 0) so the scores matmuls
    contract K=128 (sustained 64-row PE mode trips the HAM clock gate).
  - scores_T [t, s] = (K^T tile).T @ Q^T  -> softmax denominator comes from an
    appended ones-column on V (fused into the AV matmul); no max-subtraction
    (scores are O(5), exp is safe in fp32).
  - AV produces out^T [d, s] == A^T, which is exactly the lhsT the row-parallel
    o_proj needs.
  - Attention for head-pair j is emitted right after Q-projection group j, so
    the scalar-engine exp stream (the attention-phase pacer) starts during the
    projection phase instead of after it.
"""

import numpy as np
import ml_dtypes
from contextlib import ExitStack

import concourse.bass as bass
import concourse.tile as tile
from concourse import bacc, mybir
from concourse.masks import make_identity
import concourse.bass_utils as bass_utils

P = 128
S = 2048          # tokens per core (one batch element)
HID = 2048
D = 64            # head dim
NQ = 512          # 8 local q heads * 64
NKV = 128         # 2 local kv heads * 64
SW = 1024         # attention s-swath width
N_CORES = 8
BF16 = mybir.dt.bfloat16
F32 = mybir.dt.float32
AFT = mybir.ActivationFunctionType
BF = ml_dtypes.bfloat16


def _segs(sl, hi):
    """Split [sl, hi) at 512 boundaries (one PSUM bank per matmul)."""
    bounds = sorted({sl, hi} | {b for b in range(512, hi, 512) if b > sl})
    return list(zip(bounds[:-1], bounds[1:]))


def build():
    nc = bacc.Bacc("TRN2", target_bir_lowering=False, debug=False, num_devices=N_CORES)
    xt_d = nc.declare_dram_parameter("xt", [16, P, S], BF16, isOutput=False)
    wq_d = nc.declare_dram_parameter("wq", [4, P, 16 * P], BF16, isOutput=False)
    wk_d = nc.declare_dram_parameter("wk", [16, P, NKV], BF16, isOutput=False)
    wv_d = nc.declare_dram_parameter("wv", [16, P, NKV], BF16, isOutput=False)
    wo_d = nc.declare_dram_parameter("wo", [4, P, S], BF16, isOutput=False)
    cos_d = nc.declare_dram_parameter("cos2", [P, S], BF16, isOutput=False)
    sin_d = nc.declare_dram_parameter("sin2", [P, S], BF16, isOutput=False)
    mask_d = nc.declare_dram_parameter("mask", [P, P], BF16, isOutput=False)
    out_d = nc.declare_dram_parameter("out", [S, HID], F32, isOutput=True)

    with tile.TileContext(nc) as tc, ExitStack() as ctx:
        const = ctx.enter_context(tc.tile_pool(name="const", bufs=1))
        big = ctx.enter_context(tc.tile_pool(name="big", bufs=1))

        # persistent across phases
        qtz_sb = [big.tile([P, 4, S], BF16, tag=f"qtz{kv}", name=f"qtz{kv}")
                  for kv in range(2)]
        ktz_sb = [big.tile([P, S], BF16, tag=f"ktz{kv}", name=f"ktz{kv}")
                  for kv in range(2)]
        vag_sb = big.tile([P, 16, 130], BF16)    # V tiles + ones col per kv head
        at_sb = big.tile([P, 4, S], BF16)        # A^T (normalized attention out)

        cos_sb = const.tile([P, S], BF16)
        sin_sb = const.tile([P, S], BF16)
        mask_sb = const.tile([P, P], BF16)
        ident = const.tile([P, P], BF16)

        nc.sync.dma_start(cos_sb[:], cos_d[:])
        nc.sync.dma_start(sin_sb[:], sin_d[:])
        nc.sync.dma_start(mask_sb[:], mask_d[:])
        make_identity(nc, ident)
        nc.vector.memset(vag_sb[:, :, 64:65], 1.0)
        nc.vector.memset(vag_sb[:, :, 129:130], 1.0)
        for kv in range(2):
            nc.vector.memset(qtz_sb[kv][64:128, :, :], 0.0)
            nc.vector.memset(ktz_sb[kv][64:128, :], 0.0)

        # ------------- Phases 1+2 interleaved: QKV^T + RoPE + attention -------------
        with tc.tile_pool(name="ph1", bufs=1) as ph1, \
             tc.tile_pool(name="wqp", bufs=2) as wqp, \
             tc.tile_pool(name="rtmp", bufs=3) as rtmp, \
             tc.tile_pool(name="exp", bufs=12) as p_ex, \
             tc.tile_pool(name="nrm", bufs=1) as p_nrm, \
             tc.tile_pool(name="p_mm1", bufs=2, space="PSUM") as p_mm1, \
             tc.tile_pool(name="p_sc", bufs=2, space="PSUM") as p_sc, \
             tc.tile_pool(name="p_out", bufs=1, space="PSUM") as p_out:
            xt_sb = ph1.tile([P, 16, S], BF16)
            wk_sb = ph1.tile([P, 16, NKV], BF16)
            wv_sb = ph1.tile([P, 16, NKV], BF16)
            vt_sb = ph1.tile([P, S], BF16)       # V^T staging
            for i in range(16):
                nc.sync.dma_start(xt_sb[:, i, :], xt_d[i])
                nc.sync.dma_start(wk_sb[:, i, :], wk_d[i])
                nc.sync.dma_start(wv_sb[:, i, :], wv_d[i])

            def rope(ps, sw):
                # returns rope(ps) as a transient bf16 tile; ps [P,512] f32 PSUM
                cs = cos_sb[:, sw * 512:(sw + 1) * 512]
                sn = sin_sb[:, sw * 512:(sw + 1) * 512]
                q = rtmp.tile([P, 512], BF16, tag="ropeq")
                nc.scalar.copy(q[:], ps[:])
                tmp = rtmp.tile([P, 512], BF16, tag="ropet")
                for b0 in (0, 64):
                    nc.vector.tensor_copy(tmp[b0:b0 + 32, :], q[b0 + 32:b0 + 64, :])
                    nc.vector.tensor_copy(tmp[b0 + 32:b0 + 64, :], q[b0:b0 + 32, :])
                ro = rtmp.tile([P, 512], BF16, tag="ropeo")
                nc.vector.tensor_mul(ro[:], q[:], cs)
                nc.vector.tensor_mul(tmp[:], tmp[:], sn)
                nc.vector.tensor_add(ro[:], ro[:], tmp[:])
                return ro

            for sw in range(4):         # K^T
                ps = p_mm1.tile([P, 512], F32, tag="mm")
                for i in range(16):
                    nc.tensor.matmul(ps[:], wk_sb[:, i, :],
                                     xt_sb[:, i, sw * 512:(sw + 1) * 512],
                                     start=(i == 0), stop=(i == 15))
                ro = rope(ps, sw)
                for kv in range(2):
                    nc.vector.tensor_copy(ktz_sb[kv][0:64, sw * 512:(sw + 1) * 512],
                                          ro[64 * kv:64 * kv + 64, :])
            for sw in range(4):         # V^T
                ps = p_mm1.tile([P, 512], F32, tag="mm")
                for i in range(16):
                    nc.tensor.matmul(ps[:], wv_sb[:, i, :],
                                     xt_sb[:, i, sw * 512:(sw + 1) * 512],
                                     start=(i == 0), stop=(i == 15))
                nc.vector.tensor_copy(vt_sb[:, sw * 512:(sw + 1) * 512], ps[:])
            for tt in range(16):        # V^T -> V tiles (PE transpose)
                pt = p_mm1.tile([P, P], BF16, tag="mm")
                nc.tensor.transpose(pt[:], vt_sb[:, tt * 128:(tt + 1) * 128], ident[:])
                nc.vector.tensor_copy(vag_sb[:, tt, 0:64], pt[:, 0:64])
                nc.vector.tensor_copy(vag_sb[:, tt, 65:129], pt[:, 64:128])

            for j in range(4):          # Q^T block j, then attention for head-pair j
                wq_j = wqp.tile([P, 16, P], BF16, tag="wqj")
                nc.sync.dma_start(wq_j[:], wq_d[j])
                for sw in range(4):
                    ps = p_mm1.tile([P, 512], F32, tag="mm")
                    for i in range(16):
                        nc.tensor.matmul(ps[:], wq_j[:, i, :],
                                         xt_sb[:, i, sw * 512:(sw + 1) * 512],
                                         start=(i == 0), stop=(i == 15))
                    ro = rope(ps, sw)
                    for kv in range(2):
                        nc.vector.tensor_copy(
                            qtz_sb[kv][0:64, j, sw * 512:(sw + 1) * 512],
                            ro[64 * kv:64 * kv + 64, :])

                for kv in range(2):     # attention for (kv, head j)
                    pb = 64 * kv
                    vo = 65 * kv
                    for swh in range(2):
                        s0 = SW * swh
                        n_t = s0 // P + 8
                        po = p_out.tile([65, SW], F32, tag="po")
                        for tt in range(n_t):
                            t0 = P * tt
                            sl = max(0, t0 - s0)
                            sc = p_sc.tile([P, SW], F32, tag="sc")
                            for a, b in _segs(sl, SW):
                                nc.tensor.matmul(sc[:, a:b], ktz_sb[kv][:, t0:t0 + P],
                                                 qtz_sb[kv][:, j, s0 + a:s0 + b],
                                                 start=True, stop=True)
                            ex = p_ex.tile([P, SW], BF16, tag="ex")
                            nc.scalar.activation(ex[:, sl:SW], sc[:, sl:SW], AFT.Exp)
                            if t0 >= s0:  # diagonal block: zero the t>s triangle
                                nc.vector.tensor_mul(ex[:, sl:sl + P],
                                                     ex[:, sl:sl + P], mask_sb[:])
                            for a, b in _segs(sl, SW):
                                nc.tensor.matmul(po[:, a:b], vag_sb[:, tt, vo:vo + 65],
                                                 ex[:, a:b], start=(tt == 0),
                                                 stop=(tt == n_t - 1),
                                                 skip_group_check=True)
                        # one ACT copy evicts po -> SBUF so the PSUM bank frees
                        # immediately; the normalize chain then runs SBUF-side
                        pos = p_nrm.tile([65, SW], F32, tag="pos")
                        nc.vector.tensor_copy(pos[:], po[:])
                        den = p_nrm.tile([1, SW], F32, tag="den")
                        nc.vector.tensor_copy(den[:], pos[64:65, :])
                        rec = p_nrm.tile([1, SW], F32, tag="rec")
                        nc.vector.reciprocal_approx_fast(rec[:], den[:])
                        bc = p_nrm.tile([64, SW], F32, tag="bc")
                        nc.gpsimd.partition_broadcast(bc[:], rec[:], channels=64)
                        nc.vector.tensor_mul(at_sb[pb:pb + 64, j, s0:s0 + SW],
                                             pos[0:64, :], bc[:])

        # ---------------- Phase 3: o_proj (row-parallel partial) ----------------
        with tc.tile_pool(name="wop", bufs=3) as wop, \
             tc.tile_pool(name="oev", bufs=6) as p_oev, \
             tc.tile_pool(name="p_mm3", bufs=2, space="PSUM") as p_mm3:
            for nsw in range(4):
                wo_c = wop.tile([P, 4, 512], BF16, tag="woc")
                for j in range(4):
                    nc.sync.dma_start(wo_c[:, j, :], wo_d[j, :, nsw * 512:(nsw + 1) * 512])
                for tt in range(16):
                    ps = p_mm3.tile([P, 512], F32, tag="mm")
                    for j in range(4):
                        nc.tensor.matmul(ps[:], at_sb[:, j, tt * 128:(tt + 1) * 128],
                                         wo_c[:, j, :], start=(j == 0), stop=(j == 3))
                    ov = p_oev.tile([P, 512], F32, tag="ov")
                    if (tt * 4 + nsw) % 2 == 0:
                        nc.scalar.copy(ov[:], ps[:])
                    else:
                        nc.vector.tensor_copy(ov[:], ps[:])
                    nc.sync.dma_start(out_d[tt * 128:(tt + 1) * 128,
                                            nsw * 512:(nsw + 1) * 512], ov[:])

    nc.compile()
    return nc


_CACHE = {}


def _get_nc():
    if "nc" not in _CACHE:
        _CACHE["nc"] = build()
    return _CACHE["nc"]


def make_in_maps(hidden_states, Wq, Wk, Wv, Wo):
    """Host-side shard + layout prep. Returns one input map per core."""
    hs = np.asarray(hidden_states, dtype=np.float32)
    Wq = np.asarray(Wq, dtype=np.float32)
    Wk = np.asarray(Wk, dtype=np.float32)
    Wv = np.asarray(Wv, dtype=np.float32)
    Wo = np.asarray(Wo, dtype=np.float32)
    scale = D ** -0.5

    # RoPE tables, transposed ([d, t]); cos duplicated, sin signed (-,+)
    inv = 1.0 / (10000.0 ** (np.arange(0, D, 2, dtype=np.float32) / D))  # [32]
    fr = np.outer(np.arange(S, dtype=np.float32), inv)                   # [S, 32]
    cosh, sinh = np.cos(fr).T, np.sin(fr).T                              # [32, S]
    cos64 = np.concatenate([cosh, cosh], 0)                              # [64, S]
    sin64 = np.concatenate([-sinh, sinh], 0)
    cos2 = np.concatenate([cos64, cos64], 0).astype(BF)                  # [128, S]
    sin2 = np.concatenate([sin64, sin64], 0).astype(BF)
    mask = np.triu(np.ones((P, P), dtype=np.float32)).astype(BF)         # keep t <= s

    perm = [0, 4, 1, 5, 2, 6, 3, 7]  # interleave kv0/kv1 q heads per 128-block
    in_maps = []
    for c in range(N_CORES):
        b, tp = divmod(c, 4)
        xt = np.ascontiguousarray(hs[b].T).astype(BF).reshape(16, P, S)
        wq = (Wq[:, tp * NQ:(tp + 1) * NQ] * scale).reshape(HID, 8, D)
        wq = np.ascontiguousarray(wq[:, perm, :]).reshape(16, P, 4, P)
        # [4, 128, 16*128]: per head-pair j, lhsT tiles for all 16 h-blocks
        wq = np.ascontiguousarray(wq.transpose(2, 1, 0, 3)).reshape(4, P, 16 * P).astype(BF)
        wk = np.ascontiguousarray(Wk[:, tp * NKV:(tp + 1) * NKV]).astype(BF).reshape(16, P, NKV)
        wv = np.ascontiguousarray(Wv[:, tp * NKV:(tp + 1) * NKV]).astype(BF).reshape(16, P, NKV)
        wo = Wo[tp * NQ:(tp + 1) * NQ, :].reshape(8, D, HID)
        wo = np.ascontiguousarray(wo[perm]).reshape(4, P, HID).astype(BF)
        in_maps.append({"xt": xt, "wq": wq, "wk": wk, "wv": wv, "wo": wo,
                        "cos2": cos2, "sin2": sin2, "mask": mask})
    return in_maps


def kernel(hidden_states, Wq, Wk, Wv, Wo):
    nc = _get_nc()
    in_maps = make_in_maps(hidden_states, Wq, Wk, Wv, Wo)
    res = bass_utils.run_bass_kernel_spmd(nc, in_maps, list(range(N_CORES)))
    _CACHE["last_results"] = res
    parts = [res.results[c]["out"] for c in range(N_CORES)]
    out = np.stack([parts[0] + parts[1] + parts[2] + parts[3],
                    parts[4] + parts[5] + parts[6] + parts[7]], axis=0)
    return out.astype(np.float32)


# revision 48
# speedup vs baseline: 1.5355x; 1.0008x over previous
"""Trainium2 Bass kernel: GQA attention block (q/k/v proj + RoPE + causal
attention + out proj), sharded over 8 NeuronCores as DP2 (batch) x TP4 (heads).

Per core (batch b = core//4, shard tp = core%4): 2048 tokens, 8 q heads,
2 kv heads. Device computes a row-parallel partial of the output projection;
the host sums the 4 TP partials per batch.

Layout choices (all chosen so attention needs no on-chip transposes of Q/K/P):
  - X^T [hidden, tok] streamed from host (host transposes; free).
  - Q^T [d, tok] / K^T [d, tok] produced directly by the projection matmuls
    (weights as lhsT, X^T as rhs), RoPE applied in this layout, then written
    into zero-padded per-kv copies (rows 64:128 = 0) so the scores matmuls
    contract K=128 (sustained 64-row PE mode trips the HAM clock gate).
  - scores_T [t, s] = (K^T tile).T @ Q^T  -> softmax denominator comes from an
    appended ones-column on V (fused into the AV matmul); no max-subtraction
    (scores are O(5), exp is safe in fp32).
  - AV produces out^T [d, s] == A^T, which is exactly the lhsT the row-parallel
    o_proj needs.
  - Attention for head-pair j is emitted right after Q-projection group j, so
    the scalar-engine exp stream (the attention-phase pacer) starts during the
    projection phase instead of after it.
"""

import numpy as np
import ml_dtypes
from contextlib import ExitStack

import concourse.bass as bass
import concourse.tile as tile
from concourse import bacc, mybir
from concourse.masks import make_identity
import concourse.bass_utils as bass_utils

P = 128
S = 2048          # tokens per core (one batch element)
HID = 2048
D = 64            # head dim
NQ = 512          # 8 local q heads * 64
NKV = 128         # 2 local kv heads * 64
SW = 1024         # attention s-swath width
N_CORES = 8
BF16 = mybir.dt.bfloat16
F32 = mybir.dt.float32
AFT = mybir.ActivationFunctionType
BF = ml_dtypes.bfloat16


def _segs(sl, hi):
    """Split [sl, hi) at 512 boundaries (one PSUM bank per matmul)."""
    bounds = sorted({sl, hi} | {b for b in range(512, hi, 512) if b > sl})
    return list(zip(bounds[:-1], bounds[1:]))


def build():
    nc = bacc.Bacc("TRN2", target_bir_lowering=False, debug=False, num_devices=N_CORES)
    xt_d = nc.declare_dram_parameter("xt", [16, P, S], BF16, isOutput=False)
    wq_d = nc.declare_dram_parameter("wq", [4, P, 16 * P], BF16, isOutput=False)
    wk_d = nc.declare_dram_parameter("wk", [16, P, NKV], BF16, isOutput=False)
    wv_d = nc.declare_dram_parameter("wv", [16, P, NKV], BF16, isOutput=False)
    wo_d = nc.declare_dram_parameter("wo", [4, P, S], BF16, isOutput=False)
    cos_d = nc.declare_dram_parameter("cos2", [P, S], BF16, isOutput=False)
    sin_d = nc.declare_dram_parameter("sin2", [P, S], BF16, isOutput=False)
    mask_d = nc.declare_dram_parameter("mask", [P, P], BF16, isOutput=False)
    out_d = nc.declare_dram_parameter("out", [S, HID], F32, isOutput=True)

    with tile.TileContext(nc) as tc, ExitStack() as ctx:
        const = ctx.enter_context(tc.tile_pool(name="const", bufs=1))
        big = ctx.enter_context(tc.tile_pool(name="big", bufs=1))

        # persistent across phases
        qtz_sb = [big.tile([P, 4, S], BF16, tag=f"qtz{kv}", name=f"qtz{kv}")
                  for kv in range(2)]
        ktz_sb = [big.tile([P, S], BF16, tag=f"ktz{kv}", name=f"ktz{kv}")
                  for kv in range(2)]
        vag_sb = big.tile([P, 16, 130], BF16)    # V tiles + ones col per kv head
        at_sb = big.tile([P, 4, S], BF16)        # A^T (normalized attention out)

        cos_sb = const.tile([P, S], BF16)
        sin_sb = const.tile([P, S], BF16)
        mask_sb = const.tile([P, P], BF16)
        ident = const.tile([P, P], BF16)

        nc.sync.dma_start(cos_sb[:], cos_d[:])
        nc.sync.dma_start(sin_sb[:], sin_d[:])
        nc.sync.dma_start(mask_sb[:], mask_d[:])
        make_identity(nc, ident)
        nc.vector.memset(vag_sb[:, :, 64:65], 1.0)
        nc.vector.memset(vag_sb[:, :, 129:130], 1.0)
        for kv in range(2):
            nc.vector.memset(qtz_sb[kv][64:128, :, :], 0.0)
            nc.vector.memset(ktz_sb[kv][64:128, :], 0.0)

        # ------------- Phases 1+2 interleaved: QKV^T + RoPE + attention -------------
        with tc.tile_pool(name="ph1", bufs=1) as ph1, \
             tc.tile_pool(name="wqp", bufs=2) as wqp, \
             tc.tile_pool(name="rtmp", bufs=3) as rtmp, \
             tc.tile_pool(name="exp", bufs=12) as p_ex, \
             tc.tile_pool(name="nrm", bufs=1) as p_nrm, \
             tc.tile_pool(name="p_mm1", bufs=2, space="PSUM") as p_mm1, \
             tc.tile_pool(name="p_sc", bufs=2, space="PSUM") as p_sc, \
             tc.tile_pool(name="p_out", bufs=1, space="PSUM") as p_out:
            xt_sb = ph1.tile([P, 16, S], BF16)
            wk_sb = ph1.tile([P, 16, NKV], BF16)
            wv_sb = ph1.tile([P, 16, NKV], BF16)
            vt_sb = ph1.tile([P, S], BF16)       # V^T staging
            for i in range(16):
                nc.sync.dma_start(xt_sb[:, i, :], xt_d[i])
                nc.sync.dma_start(wk_sb[:, i, :], wk_d[i])
                nc.sync.dma_start(wv_sb[:, i, :], wv_d[i])

            def rope(ps, sw):
                # returns rope(ps) as a transient bf16 tile; ps [P,512] f32 PSUM
                cs = cos_sb[:, sw * 512:(sw + 1) * 512]
                sn = sin_sb[:, sw * 512:(sw + 1) * 512]
                q = rtmp.tile([P, 512], BF16, tag="ropeq")
                nc.scalar.copy(q[:], ps[:])
                tmp = rtmp.tile([P, 512], BF16, tag="ropet")
                for b0 in (0, 64):
                    nc.vector.tensor_copy(tmp[b0:b0 + 32, :], q[b0 + 32:b0 + 64, :])
                    nc.vector.tensor_copy(tmp[b0 + 32:b0 + 64, :], q[b0:b0 + 32, :])
                ro = rtmp.tile([P, 512], BF16, tag="ropeo")
                nc.vector.tensor_mul(ro[:], q[:], cs)
                nc.vector.tensor_mul(tmp[:], tmp[:], sn)
                nc.vector.tensor_add(ro[:], ro[:], tmp[:])
                return ro

            for sw in range(4):         # K^T
                ps = p_mm1.tile([P, 512], F32, tag="mm")
                for i in range(16):
                    nc.tensor.matmul(ps[:], wk_sb[:, i, :],
                                     xt_sb[:, i, sw * 512:(sw + 1) * 512],
                                     start=(i == 0), stop=(i == 15))
                ro = rope(ps, sw)
                for kv in range(2):
                    nc.vector.tensor_copy(ktz_sb[kv][0:64, sw * 512:(sw + 1) * 512],
                                          ro[64 * kv:64 * kv + 64, :])
            for sw in range(4):         # V^T
                ps = p_mm1.tile([P, 512], F32, tag="mm")
                for i in range(16):
                    nc.tensor.matmul(ps[:], wv_sb[:, i, :],
                                     xt_sb[:, i, sw * 512:(sw + 1) * 512],
                                     start=(i == 0), stop=(i == 15))
                nc.vector.tensor_copy(vt_sb[:, sw * 512:(sw + 1) * 512], ps[:])
            for tt in range(16):        # V^T -> V tiles (PE transpose)
                pt = p_mm1.tile([P, P], BF16, tag="mm")
                nc.tensor.transpose(pt[:], vt_sb[:, tt * 128:(tt + 1) * 128], ident[:])
                nc.vector.tensor_copy(vag_sb[:, tt, 0:64], pt[:, 0:64])
                nc.vector.tensor_copy(vag_sb[:, tt, 65:129], pt[:, 64:128])

            for j in range(4):          # Q^T block j, then attention for head-pair j
                wq_j = wqp.tile([P, 16, P], BF16, tag="wqj")
                nc.sync.dma_start(wq_j[:], wq_d[j])
                for sw in range(4):
                    ps = p_mm1.tile([P, 512], F32, tag="mm")
                    for i in range(16):
                        nc.tensor.matmul(ps[:], wq_j[:, i, :],
                                         xt_sb[:, i, sw * 512:(sw + 1) * 512],
                                         start=(i == 0), stop=(i == 15))
                    ro = rope(ps, sw)
                    for kv in range(2):
                        nc.vector.tensor_copy(
                            qtz_sb[kv][0:64, j, sw * 512:(sw + 1) * 512],
                            ro[64 * kv:64 * kv + 64, :])

                for kv in range(2):     # attention for (kv, head j)
                    pb = 64 * kv
                    vo = 65 * kv
                    for swh in range(2):
                        s0 = SW * swh
                        n_t = s0 // P + 8
                        po = p_out.tile([65, SW], F32, tag="po")
                        for tt in range(n_t):
                            t0 = P * tt
                            sl = max(0, t0 - s0)
                            sc = p_sc.tile([P, SW], F32, tag="sc")
                            for a, b in _segs(sl, SW):
                                nc.tensor.matmul(sc[:, a:b], ktz_sb[kv][:, t0:t0 + P],
                                                 qtz_sb[kv][:, j, s0 + a:s0 + b],
                                                 start=True, stop=True)
                            ex = p_ex.tile([P, SW], BF16, tag="ex")
                            nc.scalar.activation(ex[:, sl:SW], sc[:, sl:SW], AFT.Exp)
                            if t0 >= s0:  # diagonal block: zero the t>s triangle
                                nc.vector.tensor_mul(ex[:, sl:sl + P],
                                                     ex[:, sl:sl + P], mask_sb[:])
                            for a, b in _segs(sl, SW):
                                nc.tensor.matmul(po[:, a:b], vag_sb[:, tt, vo:vo + 65],
                                                 ex[:, a:b], start=(tt == 0),
                                                 stop=(tt == n_t - 1),
                                                 skip_group_check=True)
                        # one ACT copy evicts po -> SBUF so the PSUM bank frees
                        # immediately; the normalize chain then runs SBUF-side
                        pos = p_nrm.tile([65, SW], F32, tag="pos")
                        nc.vector.tensor_copy(pos[:], po[:])
                        den = p_nrm.tile([1, SW], F32, tag="den")
                        nc.vector.tensor_copy(den[:], pos[64:65, :])
                        rec = p_nrm.tile([1, SW], F32, tag="rec")
                        nc.vector.reciprocal_approx_fast(rec[:], den[:])
                        bc = p_nrm.tile([64, SW], F32, tag="bc")
                        nc.gpsimd.partition_broadcast(bc[:], rec[:], channels=64)
                        nc.vector.tensor_mul(at_sb[pb:pb + 64, j, s0:s0 + SW],
                                             pos[0:64, :], bc[:])

        # ---------------- Phase 3: o_proj (row-parallel partial) ----------------
        with tc.tile_pool(name="wop", bufs=3) as wop, \
             tc.tile_pool(name="oev", bufs=6) as p_oev, \
             tc.tile_pool(name="p_mm3", bufs=2, space="PSUM") as p_mm3:
            for nsw in range(4):
                wo_c = wop.tile([P, 4, 512], BF16, tag="woc")
                for j in range(4):
                    nc.sync.dma_start(wo_c[:, j, :], wo_d[j, :, nsw * 512:(nsw + 1) * 512])
                for tt in range(16):
                    ps = p_mm3.tile([P, 512], F32, tag="mm")
                    for j in range(4):
                        nc.tensor.matmul(ps[:], at_sb[:, j, tt * 128:(tt + 1) * 128],
                                         wo_c[:, j, :], start=(j == 0), stop=(j == 3))
                    ov = p_oev.tile([P, 512], F32, tag="ov")
                    if (tt * 4 + nsw) % 2 == 0:
                        nc.scalar.copy(ov[:], ps[:])
                    else:
                        nc.vector.tensor_copy(ov[:], ps[:])
                    nc.sync.dma_start(out_d[tt * 128:(tt + 1) * 128,
                                            nsw * 512:(nsw + 1) * 512], ov[:])

    nc.compile()
    return nc


_CACHE = {}


def _get_nc():
    if "nc" not in _CACHE:
        _CACHE["nc"] = build()
    return _CACHE["nc"]


def make_in_maps(hidden_states, Wq, Wk, Wv, Wo):
    """Host-side shard + layout prep. Returns one input map per core."""
    hs = np.asarray(hidden_states, dtype=np.float32)
    Wq = np.asarray(Wq, dtype=np.float32)
    Wk = np.asarray(Wk, dtype=np.float32)
    Wv = np.asarray(Wv, dtype=np.float32)
    Wo = np.asarray(Wo, dtype=np.float32)
    scale = D ** -0.5

    # RoPE tables, transposed ([d, t]); cos duplicated, sin signed (-,+)
    inv = 1.0 / (10000.0 ** (np.arange(0, D, 2, dtype=np.float32) / D))  # [32]
    fr = np.outer(np.arange(S, dtype=np.float32), inv)                   # [S, 32]
    cosh, sinh = np.cos(fr).T, np.sin(fr).T                              # [32, S]
    cos64 = np.concatenate([cosh, cosh], 0)                              # [64, S]
    sin64 = np.concatenate([-sinh, sinh], 0)
    cos2 = np.concatenate([cos64, cos64], 0).astype(BF)                  # [128, S]
    sin2 = np.concatenate([sin64, sin64], 0).astype(BF)
    mask = np.triu(np.ones((P, P), dtype=np.float32)).astype(BF)         # keep t <= s

    perm = [0, 4, 1, 5, 2, 6, 3, 7]  # interleave kv0/kv1 q heads per 128-block
    in_maps = []
    for c in range(N_CORES):
        b, tp = divmod(c, 4)
        xt = np.ascontiguousarray(hs[b].T).astype(BF).reshape(16, P, S)
        wq = (Wq[:, tp * NQ:(tp + 1) * NQ] * scale).reshape(HID, 8, D)
        wq = np.ascontiguousarray(wq[:, perm, :]).reshape(16, P, 4, P)
        # [4, 128, 16*128]: per head-pair j, lhsT tiles for all 16 h-blocks
        wq = np.ascontiguousarray(wq.transpose(2, 1, 0, 3)).reshape(4, P, 16 * P).astype(BF)
        wk = np.ascontiguousarray(Wk[:, tp * NKV:(tp + 1) * NKV]).astype(BF).reshape(16, P, NKV)
        wv = np.ascontiguousarray(Wv[:, tp * NKV:(tp + 1) * NKV]).astype(BF).reshape(16, P, NKV)
        wo = Wo[tp * NQ:(tp + 1) * NQ, :].reshape(8, D, HID)
        wo = np.ascontiguousarray(wo[perm]).reshape(4, P, HID).astype(BF)
        in_maps.append({"xt": xt, "wq": wq, "wk": wk, "wv": wv, "wo": wo,
                        "cos2": cos2, "sin2": sin2, "mask": mask})
    return in_maps


def kernel(hidden_states, Wq, Wk, Wv, Wo):
    nc = _get_nc()
    in_maps = make_in_maps(hidden_states, Wq, Wk, Wv, Wo)
    res = bass_utils.run_bass_kernel_spmd(nc, in_maps, list(range(N_CORES)))
    _CACHE["last_results"] = res
    parts = [res.results[c]["out"] for c in range(N_CORES)]
    out = np.stack([parts[0] + parts[1] + parts[2] + parts[3],
                    parts[4] + parts[5] + parts[6] + parts[7]], axis=0)
    return out.astype(np.float32)
